# revision 40
# baseline (speedup 1.0000x reference)
"""Trainium2 Bass kernel for BatchedGNNModel (4-layer GCN over 3-rod chain graph).

Contract: kernel(**inputs) takes FULL unsharded inputs (as produced by
setup_inputs) and returns the FULL (64, 768, 3) float32 output.

Sharding: pure data parallel over batch — 8 items per NeuronCore on 8 cores,
identical SPMD program, adjacency/weights replicated (marshaled on host).

Device algorithm (fast path, zero biases):
  - A_norm = T + E with T[n,m] = d[n]*d[m] for |n-m|<=1 (d = deg^-1/2) and E
    sparse (~14 entries: rigid-body couplings + rod-boundary removals).
    Aggregation A@Z runs on the Vector engine as 3 full passes
    (U = plane.*Z; two shifted adds) plus fused (mult,add) ops for E, with
    the trailing d-scale DEFERRED through the next feature matmul / relu
    (a per-node column scale commutes with both; folded on host into the
    plane and E coefficients).
  - Feature matmuls run on the PE, weights stationary, activations
    feature-major throughout (no transposes anywhere), in float32r
    (single-pass fp32 matmul mode, ~4x fp32 throughput, rel err ~1e-4).
  - L1 aggregates before its feature matmul (F=6); L2 aggregates after
    (F=128). L3+L4 exploit that feature transforms commute with node
    aggregation: out = A@(A@(z3 W4^T)), so W4 is applied first and A_norm
    twice at F=3 — no F=128 aggregation for L3 at all. Item-packed layouts:
    groups of 4 items at partition stride 32 feed K=6 row-packed matmuls
    (feat1) and M=3 column-packed matmuls (feat4).
  - PSUM evacuation IS the aggregation's scale pass (Vector tensor_mul of
    the d^2 plane against PSUM), so each aggregation is just two shifted
    adds on SBUF plus the sparse-E ops. Emission is phase-major across two
    4-item groups so the in-order PE stream never blocks on a group's
    aggregation; relu runs on Scalar, aggregation on Vector, matmuls on PE,
    overlapped by Tile.

Fallback path (nonzero biases or dense residual E): v1 dense program — all
aggregations as PE matmuls against A_norm^T / (A_norm@A_norm)^T with bias
planes; slower but fully general.

This image's walrus accepts only one sync-wait slot per instruction, so a
post-pass splits Tile's multi-wait instructions into single-wait NoOps.
"""

import os
import sys

import numpy as np

sys.path.insert(0, "/opt/trn_rl_repo")

import concourse.bass as bass
import concourse.mybir as mybir
import concourse.tile as _tile_mod
from concourse.tile import TileContext
from concourse.vector_clock import ScopedClock
from concourse.bass_utils import run_bass_kernel_spmd


def _patched_drain_and_barrier(self, tick_clock, wait_clock):
    """The nix walrus in this image only supports one sync-wait slot on a
    Drain; Tile's kernel-tail drain carries one wait per ticked semaphore.
    Split the extra waits onto single-wait nops on the same (sync) engine —
    program order makes this equivalent before the all-engine barrier."""
    drain_inst = self.nc.sync.drain()
    wait_clock.add_sem_waits(
        drain_inst.ins, ScopedClock({None: tick_clock.global_clock}))
    waits = list(drain_inst.ins.sync_info.on_wait)
    if len(waits) > 1:
        drain_inst.ins.sync_info.on_wait = [waits[0]]
        for w in waits[1:]:
            import bass_rust
            nop = self.nc.sync.nop(nofuse=True)
            si = nop.ins.sync_info
            if si is None:
                nop.ins.sync_info = bass_rust.SyncInfo(on_wait=[w], on_update=[])
            else:
                si.on_wait = [w]
    self.nc.all_engine_barrier()
    assert self.sems is not None
    popped = self.nc._tile_sem_poison_stack.pop()
    assert popped is self._sem_poison
    self.nc.clear_and_free_semaphores(list(self.sems.allocated().values()))
    self.nc.all_engine_barrier()


_tile_mod.TileContext._drain_and_barrier = _patched_drain_and_barrier


def _split_multi_waits(nc):
    """This image's walrus supports a single sync-wait slot per instruction.
    Hoist all-but-one wait of any multi-wait instruction onto single-wait
    NoOps on the same engine, placed immediately before it (same per-engine
    program order => equivalent synchronization)."""
    for f in nc.m.functions:
        for bb in f.blocks:
            insts = list(bb.instructions)
            if not any(ins.sync_info and len(ins.sync_info.on_wait) > 1
                       for ins in insts):
                continue
            new = []
            for ins in insts:
                si = ins.sync_info
                if si is not None and len(si.on_wait) > 1:
                    waits = list(si.on_wait)
                    for w in waits[:-1]:
                        new.append(mybir.InstNoOp(
                            name=nc.get_next_instruction_name(),
                            sync_info=mybir.SyncInfo(on_wait=[w], on_update=[]),
                            bass_nofuse=True,
                            engine=ins.engine,
                        ))
                    si.on_wait = [waits[-1]]
                new.append(ins)
            bb.instructions = new


def _ensure_ntff_hook():
    """The agent image's antenv lacks axon_hooks; bass_utils imports it when
    trace=True. Install a shim and, if possible, the real ctypes profiler."""
    import types
    try:
        import antenv.axon_hooks  # noqa: F401
        return
    except Exception:
        pass
    try:
        import antenv
        mod = types.ModuleType("antenv.axon_hooks")
        state = {"h": None}
        mod.set_axon_ntff_profile_hook = lambda h: state.__setitem__("h", h)
        mod.get_axon_ntff_profile_hook = lambda: state["h"]
        sys.modules["antenv.axon_hooks"] = mod
        antenv.axon_hooks = mod
        try:
            from trn_agent_boot.trn_boot import _ntff_profile_via_ctypes
            mod.set_axon_ntff_profile_hook(
                _ntff_profile_via_ctypes("/opt/axon/libaxon_pjrt.so"))
        except Exception:
            pass
    except Exception:
        pass


_ensure_ntff_hook()

F32 = mybir.dt.float32
RELU = mybir.ActivationFunctionType.Relu

B = 64
NV = 256
N = 3 * NV  # 768
NCORES = 8
IPC = B // NCORES  # 8 items per core
KT = N // 128      # 6 node K-tiles

LAST_RUN_INFO = {}


def _build_program(with_bias: bool):
    nc = bass.Bass()

    xT_d = nc.declare_dram_parameter("xT", [IPC, 6, N], F32, isOutput=False)
    anT_d = nc.declare_dram_parameter("anT", [N, N], F32, isOutput=False)
    a2T_d = nc.declare_dram_parameter("a2T", [N, N], F32, isOutput=False)
    w1T_d = nc.declare_dram_parameter("w1T", [6, 256], F32, isOutput=False)
    w2Tp_d = nc.declare_dram_parameter("w2Tp", [128, 256], F32, isOutput=False)
    w34T_d = nc.declare_dram_parameter("w34T", [128, 3], F32, isOutput=False)
    if with_bias:
        p1t_d = nc.declare_dram_parameter("p1t", [128, 2 * N], F32, isOutput=False)
        p2t_d = nc.declare_dram_parameter("p2t", [128, N], F32, isOutput=False)
        cpt_d = nc.declare_dram_parameter("cpt", [3 * IPC, N], F32, isOutput=False)
    out_d = nc.declare_dram_parameter("outp", [3 * IPC, N], F32, isOutput=True)

    with TileContext(nc) as tc:
        with (
            tc.tile_pool(name="const", bufs=1) as cpool,
            tc.tile_pool(name="acts", bufs=2) as apool,
            tc.tile_pool(name="psf", bufs=2, space="PSUM") as psf,
            tc.tile_pool(name="psa", bufs=3, space="PSUM") as psa,
        ):
            anT = cpool.tile([128, KT * N], F32)  # [p, k*768 + j]
            nc.sync.dma_start(
                anT[:, :].rearrange("p (k j) -> p k j", j=N),
                anT_d[:, :].rearrange("(k p) j -> p k j", p=128))
            a2T = cpool.tile([128, KT * N], F32)
            nc.sync.dma_start(
                a2T[:, :].rearrange("p (k j) -> p k j", j=N),
                a2T_d[:, :].rearrange("(k p) j -> p k j", p=128))
            w1T = cpool.tile([6, 256], F32)
            nc.sync.dma_start(w1T[:, :], w1T_d[:, :])
            w2Tp = cpool.tile([128, 256], F32)
            nc.sync.dma_start(w2Tp[:, :], w2Tp_d[:, :])
            w34T = cpool.tile([128, 3], F32)
            nc.sync.dma_start(w34T[:, :], w34T_d[:, :])
            if with_bias:
                p1t = cpool.tile([128, 2 * N], F32)
                nc.sync.dma_start(p1t[:, :], p1t_d[:, :])
                p2t = cpool.tile([128, N], F32)
                nc.sync.dma_start(p2t[:, :], p2t_d[:, :])
                cpt = cpool.tile([3 * IPC, N], F32)
                nc.sync.dma_start(cpt[:, :], cpt_d[:, :])

            # Z34 for all items: [p, k*3*IPC + it*3 + f]
            z34 = cpool.tile([128, KT * 3 * IPC], F32)

            for it in range(IPC):
                xT = apool.tile([6, N], F32, tag="xT")
                nc.sync.dma_start(xT[:, :], xT_d[it])

                # feat1: Z1[node, fo] = sum_fi xT[fi, node] * W1T[fi, fo]
                z1 = apool.tile([128, KT * 256], F32, tag="z1")  # [p, m*256 + fo]
                for m in range(KT):
                    ps = psf.tile([128, 256], F32, tag="feat")
                    nc.tensor.matmul(
                        ps[:, :], xT[:, m * 128:(m + 1) * 128], w1T[:, :],
                        start=True, stop=True,
                    )
                    nc.vector.tensor_copy(z1[:, m * 256:(m + 1) * 256], ps[:, :])

                # agg1: H1t[f, j] = relu(sum_k Z1[k, f] * AnT[k, j] (+ s x b1))
                h1t = apool.tile([128, 2 * N], F32, tag="h1t")  # [fi, fh*768 + n]
                for fh in range(2):
                    for ns in range(2):
                        ps = psa.tile([128, 384], F32, tag="agg")
                        for k in range(KT):
                            nc.tensor.matmul(
                                ps[:, :],
                                z1[:, k * 256 + fh * 128: k * 256 + fh * 128 + 128],
                                anT[:, k * N + ns * 384: k * N + ns * 384 + 384],
                                start=(k == 0), stop=(k == KT - 1),
                            )
                        dst = h1t[:, fh * N + ns * 384: fh * N + ns * 384 + 384]
                        if with_bias:
                            nc.vector.tensor_tensor(
                                dst, ps[:, :],
                                p1t[:, fh * N + ns * 384: fh * N + ns * 384 + 384],
                                op=mybir.AluOpType.add,
                            )
                            nc.scalar.activation(dst, dst, RELU)
                        else:
                            nc.scalar.activation(dst, ps[:, :], RELU)

                # feat2: Z2[node, fo] = sum_fi H1t[fi, node] * W2T[fi, fo]
                z2 = apool.tile([128, KT * 128], F32, tag="z2")  # [p, m*128 + fo]
                for m in range(KT):
                    ps = psf.tile([128, 128], F32, tag="feat")
                    for kh in range(2):
                        nc.tensor.matmul(
                            ps[:, :],
                            h1t[:, kh * N + m * 128: kh * N + m * 128 + 128],
                            w2Tp[:, kh * 128:(kh + 1) * 128],
                            start=(kh == 0), stop=(kh == 1),
                        )
                    nc.vector.tensor_copy(z2[:, m * 128:(m + 1) * 128], ps[:, :])

                # agg2 + relu -> H2t (feature-major, 128 x 768)
                h2t = apool.tile([128, N], F32, tag="h2t")
                for ns in range(2):
                    ps = psa.tile([128, 384], F32, tag="agg")
                    for k in range(KT):
                        nc.tensor.matmul(
                            ps[:, :],
                            z2[:, k * 128:(k + 1) * 128],
                            anT[:, k * N + ns * 384: k * N + ns * 384 + 384],
                            start=(k == 0), stop=(k == KT - 1),
                        )
                    dst = h2t[:, ns * 384: ns * 384 + 384]
                    if with_bias:
                        nc.vector.tensor_tensor(
                            dst, ps[:, :], p2t[:, ns * 384: ns * 384 + 384],
                            op=mybir.AluOpType.add,
                        )
                        nc.scalar.activation(dst, dst, RELU)
                    else:
                        nc.scalar.activation(dst, ps[:, :], RELU)

                # feat34: Z34[node, f] = sum_fi H2t[fi, node] * W34T[fi, f]
                for m in range(KT):
                    ps = psf.tile([128, 3], F32, tag="feat")
                    nc.tensor.matmul(
                        ps[:, :], h2t[:, m * 128:(m + 1) * 128], w34T[:, :],
                        start=True, stop=True,
                    )
                    base = m * 3 * IPC + it * 3
                    nc.vector.tensor_copy(z34[:, base: base + 3], ps[:, :])

            # final aggregation with A2 for all items at once
            outT = cpool.tile([3 * IPC, N], F32)
            for ns in range(2):
                ps = psa.tile([3 * IPC, 384], F32, tag="agg")
                for k in range(KT):
                    nc.tensor.matmul(
                        ps[:, :],
                        z34[:, k * 3 * IPC:(k + 1) * 3 * IPC],
                        a2T[:, k * N + ns * 384: k * N + ns * 384 + 384],
                        start=(k == 0), stop=(k == KT - 1),
                    )
                dst = outT[:, ns * 384: ns * 384 + 384]
                if with_bias:
                    nc.vector.tensor_tensor(
                        dst, ps[:, :], cpt[:, ns * 384: ns * 384 + 384],
                        op=mybir.AluOpType.add,
                    )
                else:
                    nc.vector.tensor_copy(dst, ps[:, :])
            nc.sync.dma_start(out_d[:, :], outT[:, :])

    return nc


MULT = mybir.AluOpType.mult
ADD = mybir.AluOpType.add


def _build_program_v2(ent_l1, ent_mid, ent_out):
    """Fast path. All activations feature-major; PE does weights-stationary
    feature matmuls; aggregation with A_norm runs on the vector engine:
      A_norm = T + E,  T[n,m] = d[n]*d[m] for |n-m|<=1,  E sparse.
    The trailing d-scale of each aggregation is deferred through the next
    feature matmul / relu (a per-node column scale commutes with both, d>=0),
    so each aggregation is 3 full DVE passes:
      U = plane .* Z;  S[n] = U[n-1]+U[n]+U[n+1];  plus sparse E ops.
    ent_*: (j, k, c) lists with coefficients pre-adjusted for the deferral.
    L2..L4 are emitted per 4-item group so PE/DVE/ACT pipeline across groups.
    """
    nc = bass.Bass()
    W = IPC * N  # 6144
    COPYF = mybir.ActivationFunctionType.Copy
    F32R = mybir.dt.float32r  # single-pass fp32 matmul mode

    xpk_d = nc.declare_dram_parameter("xpk", [2, 128, N], F32, isOutput=False)
    dpl_d = nc.declare_dram_parameter("dpl", [128, N], F32, isOutput=False)
    dp2_d = nc.declare_dram_parameter("dp2", [128, N], F32, isOutput=False)
    dp2g_d = nc.declare_dram_parameter("dp2g", [128, 4 * N], F32, isOutput=False)
    w1rep_d = nc.declare_dram_parameter("w1rep", [128, 256], F32R, isOutput=False)
    w2Tp_d = nc.declare_dram_parameter("w2Tp", [128, 256], F32R, isOutput=False)
    w3T_d = nc.declare_dram_parameter("w3T", [128, 128], F32R, isOutput=False)
    w4T_d = nc.declare_dram_parameter("w4T", [128, 3], F32, isOutput=False)
    out_d = nc.declare_dram_parameter("outp", [2, 128, N], F32, isOutput=True)

    with TileContext(nc) as tc:
        with (
            tc.tile_pool(name="const", bufs=1) as cpool,
            tc.tile_pool(name="acts", bufs=1) as apool,
            tc.tile_pool(name="grp", bufs=2) as gpool,
            tc.tile_pool(name="ps1", bufs=4, space="PSUM") as ps1,
            tc.tile_pool(name="ps2", bufs=4, space="PSUM") as ps2,
        ):
            dpl = cpool.tile([128, N], F32)
            nc.sync.dma_start(dpl[:, :], dpl_d[:, :])
            dp2 = cpool.tile([128, N], F32)
            nc.sync.dma_start(dp2[:, :], dp2_d[:, :])
            dp2g = cpool.tile([128, 4 * N], F32)
            nc.sync.dma_start(dp2g[:, :], dp2g_d[:, :])
            w1rep = cpool.tile([128, 256], F32R)
            nc.sync.dma_start(w1rep[:, :], w1rep_d[:, :])
            w2Tp = cpool.tile([128, 256], F32R)
            nc.sync.dma_start(w2Tp[:, :], w2Tp_d[:, :])
            w3T = cpool.tile([128, 128], F32R)
            nc.sync.dma_start(w3T[:, :], w3T_d[:, :])
            w4T = cpool.tile([128, 3], F32)
            nc.sync.dma_start(w4T[:, :], w4T_d[:, :])

            def tri(Z, H, U, zb, b, wid, plane, P=128, ubase=None,
                    eng=None):
                """S-part of one aggregation on flat tiles: windows
                Z[:, zb:], H[:, b:], U[:, u:] of width wid.
                U = plane.*Z;  H[n] = U[n-1]+U[n]+U[n+1] (in-window)."""
                e = eng if eng is not None else nc.vector
                u = b if ubase is None else ubase
                dv = plane[0:P, 0:wid]
                e.tensor_mul(U[0:P, u:u + wid], dv, Z[0:P, zb:zb + wid])
                e.tensor_add(H[0:P, b + 1:b + wid],
                             U[0:P, u + 1:u + wid],
                             U[0:P, u:u + wid - 1])
                e.tensor_copy(H[0:P, b:b + 1], U[0:P, u:u + 1])
                e.tensor_add(H[0:P, b:b + wid - 1],
                             H[0:P, b:b + wid - 1],
                             U[0:P, u + 1:u + wid])

            def tri_sum(U, H, ub, b, wid, P=128):
                """Shift-sum only (U already plane-scaled):
                H[n] = U[n-1]+U[n]+U[n+1] in-window."""
                nc.vector.tensor_add(H[0:P, b + 1:b + wid],
                                     U[0:P, ub + 1:ub + wid],
                                     U[0:P, ub:ub + wid - 1])
                nc.vector.tensor_copy(H[0:P, b:b + 1], U[0:P, ub:ub + 1])
                nc.vector.tensor_add(H[0:P, b:b + wid - 1],
                                     H[0:P, b:b + wid - 1],
                                     U[0:P, ub + 1:ub + wid])

            def ent_cols(Z, H, ents, zb=0, b=0, P=128):
                for (j, k, c) in ents:
                    nc.vector.scalar_tensor_tensor(
                        H[0:P, b + j:b + j + 1], Z[0:P, zb + k:zb + k + 1],
                        float(c), H[0:P, b + j:b + j + 1], op0=MULT, op1=ADD)

            def ent_group(Z, H, ents):
                zv = Z[:, :].rearrange("p (i n) -> p i n", n=N)
                hv = H[:, :].rearrange("p (i n) -> p i n", n=N)
                for (j, k, c) in ents:
                    nc.vector.scalar_tensor_tensor(
                        hv[:, :, j:j + 1], zv[:, :, k:k + 1], float(c),
                        hv[:, :, j:j + 1], op0=MULT, op1=ADD)

            # PE warm-up burst: fills the otherwise-idle PE window while
            # the x DMA + L1 aggregation chain runs on Sync/Vector
            for _ in range(12):
                ps = ps2.tile([128, 384], F32, tag="f2")
                nc.tensor.matmul(ps[:, 0:256], w2Tp[:, 0:128], w2Tp[:, :],
                                 start=True, stop=True)

            # ---- L1: G' = (unscaled) aggregation of x;  true G = d .* G' ----
            G = []
            for g in range(2):
                Xg = apool.tile([128, N], F32, tag=f"xg{g}")
                nc.sync.dma_start(Xg[:, :], xpk_d[g])
                Gg = apool.tile([128, N], F32R, tag=f"gg{g}")
                Ug = apool.tile([128, N], F32, tag=f"ug{g}")
                tri(Xg, Gg, Ug, 0, 0, N, dpl)
                ent_cols(Xg, Gg, ent_l1)
                G.append(Gg)

            # ---- feat1 (K=6 row-packed, weights stationary) + relu ----
            h1a = apool.tile([128, W], F32R, tag="h1a")
            h1b = apool.tile([128, W], F32R, tag="h1b")
            H1 = [h1a, h1b]
            for g in range(2):
                for half in range(2):
                    for ns in range(2):
                        for j in range(4):
                            it = g * 4 + j
                            ps = ps1.tile([128, 384], F32, tag="f1")
                            nc.tensor.matmul(
                                ps[:, :],
                                w1rep[32 * j:32 * j + 6,
                                      half * 128:(half + 1) * 128],
                                G[g][32 * j:32 * j + 6,
                                     ns * 384:(ns + 1) * 384],
                                start=True, stop=True,
                                tile_position=(32 * j, 0))
                            nc.scalar.activation(
                                H1[half][:, it * N + ns * 384:
                                         it * N + (ns + 1) * 384],
                                ps[:, :], RELU)

            def feat2(gi, Z2g):
                base = gi * 4 * N
                for c6 in range(6):
                    ps = ps2.tile([128, 512], F32, tag="f2")
                    for kh in range(2):
                        nc.tensor.matmul(
                            ps[:, :], w2Tp[:, kh * 128:(kh + 1) * 128],
                            H1[kh][:, base + c6 * 512: base + (c6 + 1) * 512],
                            start=(kh == 0), stop=(kh == 1))
                    nc.vector.tensor_mul(
                        Z2g[:, c6 * 512:(c6 + 1) * 512],
                        dp2g[:, c6 * 512:(c6 + 1) * 512], ps[:, :])

            def agg(Zg, Hg, relu):
                for j4 in range(4):
                    tri_sum(Zg, Hg, j4 * N, j4 * N, N)
                ent_group(Zg, Hg, ent_mid)
                if relu:
                    for j4 in range(4):
                        nc.scalar.activation(Hg[:, j4 * N:(j4 + 1) * N],
                                             Hg[:, j4 * N:(j4 + 1) * N], RELU)

            def feat2_agg(gi, Z2g, H2g):
                base = gi * 4 * N
                done = 0
                for c6 in range(6):
                    ps = ps2.tile([128, 512], F32, tag="f2")
                    for kh in range(2):
                        nc.tensor.matmul(
                            ps[:, :], w2Tp[:, kh * 128:(kh + 1) * 128],
                            H1[kh][:, base + c6 * 512: base + (c6 + 1) * 512],
                            start=(kh == 0), stop=(kh == 1))
                    nc.vector.tensor_mul(
                        Z2g[:, c6 * 512:(c6 + 1) * 512],
                        dp2g[:, c6 * 512:(c6 + 1) * 512], ps[:, :])
                    while done < 4 and (done + 1) * N <= (c6 + 1) * 512:
                        tri_sum(Z2g, H2g, done * N, done * N, N)
                        done += 1
                while done < 4:
                    tri_sum(Z2g, H2g, done * N, done * N, N)
                    done += 1
                ent_group(Z2g, H2g, ent_mid)
                for j4 in range(4):
                    nc.scalar.activation(H2g[:, j4 * N:(j4 + 1) * N],
                                         H2g[:, j4 * N:(j4 + 1) * N], RELU)

            def feat3(gi, H2g, Z3g):
                for c6 in range(6):
                    ps = ps2.tile([128, 512], F32, tag="f2")
                    nc.tensor.matmul(
                        ps[:, :], w3T[:, :],
                        H2g[:, c6 * 512:(c6 + 1) * 512],
                        start=True, stop=True)
                    nc.scalar.activation(
                        Z3g[:, c6 * 512:(c6 + 1) * 512], ps[:, :], COPYF)

            def feat4_agg4(gi, Z3g):
                # W4 first (features commute with node aggregation), then
                # A_norm applied twice at F=3 — replaces the F=128 L3 agg
                U4 = gpool.tile([128, N], F32, tag="g4")
                for ns in range(2):
                    ps = ps2.tile([128, 384], F32, tag="f2")
                    for j in range(4):
                        nc.tensor.matmul(
                            ps[32 * j:32 * j + 3, :], w4T[:, :],
                            Z3g[:, j * N + ns * 384: j * N + (ns + 1) * 384],
                            start=True, stop=True,
                            tile_position=(0, 32 * j))
                    nc.vector.tensor_mul(
                        U4[:, ns * 384:(ns + 1) * 384],
                        dp2[:, ns * 384:(ns + 1) * 384], ps[:, :])
                P4 = gpool.tile([128, N], F32, tag="o4")
                tri_sum(U4, P4, 0, 0, N)
                ent_cols(U4, P4, ent_mid)
                U5 = gpool.tile([128, N], F32, tag="u5")
                nc.vector.tensor_mul(U5[:, :], dp2[:, :], P4[:, :])
                O4 = gpool.tile([128, N], F32, tag="o5")
                tri_sum(U5, O4, 0, 0, N)
                ent_cols(U5, O4, ent_mid)
                nc.vector.tensor_mul(O4[:, :], dpl[:, :], O4[:, :])
                nc.sync.dma_start(out_d[gi], O4[:, :])

            # phase-major emission: the PE stream never blocks on a group's
            # aggregation — the other group's feature matmuls come first
            Z2a = gpool.tile([128, 4 * N], F32, tag="tagZ")
            H2a = gpool.tile([128, 4 * N], F32R, tag="tagH")
            feat2_agg(0, Z2a, H2a)
            Z2b = gpool.tile([128, 4 * N], F32, tag="tagZ")
            H2b = gpool.tile([128, 4 * N], F32R, tag="tagH")
            feat2_agg(1, Z2b, H2b)
            Z3a = gpool.tile([128, 4 * N], F32, tag="tagZ")
            Z3b = gpool.tile([128, 4 * N], F32, tag="tagZ")
            feat3(0, H2a, Z3a)
            feat4_agg4(0, Z3a)
            feat3(1, H2b, Z3b)
            feat4_agg4(1, Z3b)

    return nc


def kernel(x, inputs, adjacency, W1, b1, W2, b2, W3, b3, W4, b4,
           parent_sel, child1_sel, child2_sel):
    global LAST_RUN_INFO
    x = np.asarray(x, np.float32)
    inp = np.asarray(inputs, np.float32)
    A = np.asarray(adjacency, np.float32)
    W1 = np.asarray(W1, np.float32); b1 = np.asarray(b1, np.float32)
    W2 = np.asarray(W2, np.float32); b2 = np.asarray(b2, np.float32)
    W3 = np.asarray(W3, np.float32); b3 = np.asarray(b3, np.float32)
    W4 = np.asarray(W4, np.float32); b4 = np.asarray(b4, np.float32)
    parent_sel = np.asarray(parent_sel, np.int64)
    child1_sel = np.asarray(child1_sel, np.int64)
    child2_sel = np.asarray(child2_sel, np.int64)

    # ---- host prep (replicated constants + layout marshaling) ----
    # clamp rows in global node index space
    clamp_rows = np.concatenate([
        parent_sel, NV + child1_sel, 2 * NV + child2_sel,
    ]).astype(np.int64)

    x0 = x.copy()
    x0[:, clamp_rows, 0:3] = inp[:, clamp_rows, :]

    deg = A.sum(axis=-1)
    deg_safe = np.where(deg == 0, np.float32(1.0), deg)
    d = np.where(deg == 0, np.float32(0.0), deg_safe ** np.float32(-0.5)).astype(np.float32)
    A_norm = (A * d[:, None] * d[None, :]).astype(np.float32)
    AnT = np.ascontiguousarray(A_norm.T)
    A2T = np.ascontiguousarray((A_norm @ A_norm).T.astype(np.float32))

    W1T = np.ascontiguousarray(W1.T)                       # (6, 256)
    W2Tp = np.ascontiguousarray(                           # (128, 256): [p, kh*128+f]
        W2.T.reshape(2, 128, 128).transpose(1, 0, 2).reshape(128, 256))
    W34T = np.ascontiguousarray(W3.T @ W4.T)               # (128, 3)

    with_bias = bool(np.any(b1) or np.any(b2) or np.any(b3) or np.any(b4))
    extra = {}
    if with_bias:
        s = A_norm.sum(axis=1).astype(np.float32)          # A_norm @ 1
        s2 = (A_norm @ s).astype(np.float32)
        # P1t[fi, fh*768 + n] = b1[fh*128+fi] * s[n]
        p1t = np.einsum('f,n->fn', b1, s).astype(np.float32)        # (256, 768)
        p1t = p1t.reshape(2, 128, N).transpose(1, 0, 2).reshape(128, 2 * N)
        p2t = np.einsum('f,n->fn', b2, s).astype(np.float32)        # (128, 768)
        cp = (np.einsum('f,n->fn', W4 @ b3, s2) +
              np.einsum('f,n->fn', b4, s)).astype(np.float32)       # (3, 768)
        cpt = np.tile(cp, (IPC, 1)).astype(np.float32)              # (24, 768)
        extra = {"p1t": np.ascontiguousarray(p1t),
                 "p2t": np.ascontiguousarray(p2t),
                 "cpt": np.ascontiguousarray(cpt)}

    # sparse residual of A_norm vs the tridiagonal d-outer-product model
    E = A_norm.copy()
    idx = np.arange(N)
    for o in (-1, 0, 1):
        n = idx[max(0, -o):N - max(0, o)]
        E[n, n + o] -= (d[n] * d[n + o]).astype(np.float32)
    nz = np.argwhere(E != 0)
    entries = [(int(j), int(k), float(E[j, k])) for j, k in nz]

    use_v2 = (not with_bias) and len(entries) <= 96

    if use_v2:
        # item-packed inputs: 2 groups of 4 items at partition stride 32
        xpk = np.zeros((NCORES, 2, 128, N), np.float32)
        for c in range(NCORES):
            for g in range(2):
                for j in range(4):
                    xpk[c, g, 32 * j:32 * j + 6, :] = \
                        x0[c * IPC + g * 4 + j].T
        dpl = np.ascontiguousarray(
            np.broadcast_to(d, (128, N)).astype(np.float32))
        dp2 = np.ascontiguousarray(
            np.broadcast_to((d * d).astype(np.float32), (128, N)))
        dp2g = np.ascontiguousarray(
            np.broadcast_to(np.tile((d * d).astype(np.float32), 4),
                            (128, 4 * N)))
        w1rep = np.zeros((128, 256), np.float32)
        for j in range(4):
            w1rep[32 * j:32 * j + 6, :] = W1T
        w3T = np.ascontiguousarray(W3.T)
        w4T = np.ascontiguousarray(W4.T)

        # entry coefficients adjusted for the deferred d-scale
        dj = np.where(d == 0, np.float32(1.0), d)
        ent_l1 = [(j, k, c / float(dj[j])) for (j, k, c) in entries]
        ent_mid = [(j, k, c / (float(dj[j]) * float(dj[k])))
                   for (j, k, c) in entries]
        ent_out = [(j, k, c / float(dj[k])) for (j, k, c) in entries]

        nc = _build_program_v2(ent_l1, ent_mid, ent_out)
        _split_multi_waits(nc)
        in_maps = [{
            "xpk": xpk[c], "dpl": dpl, "dp2": dp2, "dp2g": dp2g,
            "w1rep": w1rep,
            "w2Tp": W2Tp, "w3T": w3T, "w4T": w4T,
        } for c in range(NCORES)]
    else:
        # per-core input shards: xT[core][it] = x0[core*IPC+it].T  (6, 768)
        xT_all = np.ascontiguousarray(
            x0.transpose(0, 2, 1).reshape(NCORES, IPC, 6, N))

        nc = _build_program(with_bias)
        _split_multi_waits(nc)

        in_maps = []
        for c in range(NCORES):
            m = {
                "xT": xT_all[c],
                "anT": AnT,
                "a2T": A2T,
                "w1T": W1T,
                "w2Tp": W2Tp,
                "w34T": W34T,
            }
            m.update(extra)
            in_maps.append(m)

    trace = os.environ.get("KERNEL_TRACE", "") == "1"
    res = run_bass_kernel_spmd(nc, in_maps, list(range(NCORES)), trace=trace)

    LAST_RUN_INFO = {
        "exec_time_ns": res.exec_time_ns,
        "mean_exec_time_ns": res.mean_exec_time_ns,
        "max_exec_time_core_id": res.max_exec_time_core_id,
    }

    out = np.empty((B, N, 3), np.float32)
    for c in range(NCORES):
        o = res.results[c]["outp"]
        if use_v2:  # (2, 128, 768), item g*4+j at partitions 32j..32j+3
            for g in range(2):
                for j in range(4):
                    out[c * IPC + g * 4 + j] = o[g, 32 * j:32 * j + 3, :].T
        else:       # (24, 768)
            for it in range(IPC):
                out[c * IPC + it] = o[it * 3:(it + 1) * 3, :].T
    # output clamp
    out[:, clamp_rows, :] = inp[:, clamp_rows, :]
    return out
